# revision 3
# baseline (speedup 1.0000x reference)
"""CenterLoss kernel for 8 Trainium2 NeuronCores.

loss = mean(distmat * onehot(labels)) over a (B, C) distmat where
distmat[i, j] = ||x_i - c_j||^2.  The mask selects exactly one element
per row, so  loss = (1/(B*C)) * sum_i ||x_i - c_{labels[i]}||^2.

Strategy: data-parallel over batch.  Each of the 8 cores takes 512 rows
of x and gathers its 512 center rows from the (replicated) centers
table with a SINGLE dma_gather (InstDMAGatherAnt, mlp gpsimd library):
one Q7 descriptor-generation pass (~994ns fixed + 0.34ns/desc) instead
of 4 serial indirect DMAs (~1µs fixed each).  The x load is issued on
the SWDGE queue first so it absorbs the one-time Q7 SWDGE startup cost
while the (HWDGE) index load is still in flight.  The vector engine
then does one [128, 4*128] subtract and one fused square+accumulate,
and a [128,1] partial-sum tile goes back to HBM.  Host sums in float64
and divides by B*C.

Layouts (host-prepared, see make_in_maps):
  dma_gather writes gathered row i to partition i%128, slot i//128, so
  x is pre-reordered to x_dev[p, n, :] = x_shard[n*128 + p].
  Indices are int16 (labels < 20000 < 32767), wrapped as
  idx[p, s] = labels_shard[s*16 + p%16]  (hardware reads partitions
  0-15; rows 16-127 replicated so every partition holds valid indices).
"""

import sys

if "/opt/trn_rl_repo" not in sys.path:
    sys.path.insert(0, "/opt/trn_rl_repo")

import numpy as np

import concourse.bass as bass
from concourse import mybir
from concourse.library_config import mlp

NCORES = 8
B = 4096
D = 128
C = 20000
P = 128
BS = B // NCORES          # 512 rows per core
N = BS // P               # 4 slots per partition


def build_bass() -> bass.Bass:
    nc = bass.Bass(num_swdge_queues=2)
    x = nc.declare_dram_parameter("x", [P, N, D], mybir.dt.float32, isOutput=False)
    idx = nc.declare_dram_parameter("idx16", [P, BS // 16], mybir.dt.int16,
                                    isOutput=False)
    centers = nc.declare_dram_parameter(
        "centers", [C, D], mybir.dt.float32, isOutput=False
    )
    out = nc.declare_dram_parameter("out", [P, 1], mybir.dt.float32, isOutput=True)

    with (
        nc.sbuf_tensor([P, BS // 16], mybir.dt.int16) as idx_t,
        nc.sbuf_tensor([P, N, D], mybir.dt.float32) as x_t,
        nc.sbuf_tensor([P, N, D], mybir.dt.float32) as g_t,
        nc.sbuf_tensor([P, N, D], mybir.dt.float32) as d_t,
        nc.sbuf_tensor([P, N, D], mybir.dt.float32) as sq_t,
        nc.sbuf_tensor([P, 1], mybir.dt.float32) as red_t,
        nc.semaphore("idx_sem") as idx_sem,
        nc.semaphore("x_sem") as x_sem,
        nc.semaphore("g_sem") as g_sem,
        nc.semaphore("v_sem") as v_sem,
        nc.semaphore("done_sem") as done_sem,
    ):
        # Index load on HWDGE (Sync) — issued first thing after the
        # NEFF preamble; its ~2µs completion latency overlaps the
        # SWDGE warmup below.
        idx_dma = nc.sync.dma_start(out=idx_t[:], in_=idx[:])
        idx_dma.ins.single_packet = True
        idx_dma.then_inc(idx_sem, 16)

        with nc.Block(no_gpsimd_drain=True) as block:

            @block.sync
            def _(sync):
                sync.wait_ge(v_sem, 2)
                # Sync-queue drain at block end guarantees the store
                # lands before kernel completion.
                out_dma = sync.dma_start(out=out[:], in_=red_t[:])
                out_dma.ins.single_packet = True
                out_dma.then_inc(done_sem, 16)

            @block.gpsimd
            def _(gpsimd):
                # x load via SWDGE: absorbs the one-time Q7 startup
                # (~1µs) while the idx DMA is still in flight, and
                # needs no extra engine.
                gpsimd.dma_start(out=x_t[:], in_=x[:]).then_inc(x_sem, 16)
                # dma_gather lives in the mlp gpsimd library.
                gpsimd.load_library(mlp)
                gpsimd.wait_ge(idx_sem, 16)
                gpsimd.dma_gather(
                    g_t[:, :, :],
                    centers[:],
                    idx_t[:],
                    BS,            # num_idxs
                    BS,            # num_idxs_reg (all indices valid)
                    D,             # elem_size (fp32 elements = 512B rows)
                    queue_num=1,   # keep gather ring separate from x load
                ).then_inc(g_sem, 16)

            @block.vector
            def _(vector):
                vector.wait_ge(x_sem, 16)
                vector.wait_ge(g_sem, 16)
                vector.tensor_tensor(
                    out=d_t[:, :, :],
                    in0=x_t[:, :, :],
                    in1=g_t[:, :, :],
                    op=mybir.AluOpType.subtract,
                ).then_inc(v_sem, 1)
                vector.wait_ge(v_sem, 1)
                # sq = (d + 0) * d ; accum = sum(sq) — fused square+reduce
                vector.scalar_tensor_tensor(
                    out=sq_t[:, :, :],
                    in0=d_t[:, :, :],
                    scalar=0.0,
                    in1=d_t[:, :, :],
                    op0=mybir.AluOpType.add,
                    op1=mybir.AluOpType.mult,
                    accum_out=red_t[:, 0:1],
                ).then_inc(v_sem, 1)

    # Raw Bass skips Bacc's codegen_inst_isa_subclasses pass; without it
    # the load_library InstISA has empty .instr bytes and walrus fails
    # with "ISA wrong length".
    from concourse.library_overlay import lower_extended_insts

    lower_extended_insts(nc)
    if not nc.is_finalized():
        nc.finalize()
    return nc


_NC = None


def _get_nc() -> bass.Bass:
    global _NC
    if _NC is None:
        _NC = build_bass()
    return _NC


def make_in_maps(x, labels, centers):
    x = np.ascontiguousarray(np.asarray(x, dtype=np.float32))
    labels = np.asarray(labels).astype(np.int64)
    centers = np.ascontiguousarray(np.asarray(centers, dtype=np.float32))
    in_maps = []
    for c in range(NCORES):
        sl = slice(c * BS, (c + 1) * BS)
        xs = x[sl]
        # dma_gather puts row i at (partition i%128, slot i//128);
        # reorder x to match: x_dev[p, n, :] = xs[n*128 + p].
        x_dev = np.ascontiguousarray(
            xs.reshape(N, P, D).transpose(1, 0, 2)
        )
        lab = labels[sl].astype(np.int16)
        # wrapped int16 index layout: idx[p, s] = lab[s*16 + p%16],
        # replicated across all 8 groups of 16 partitions.
        idx16 = np.ascontiguousarray(
            np.tile(lab.reshape(BS // 16, 16).T, (P // 16, 1))
        )
        in_maps.append(
            {
                "x": x_dev,
                "idx16": idx16,
                "centers": centers,
            }
        )
    return in_maps


def reduce_outputs(results) -> np.ndarray:
    total = 0.0
    for r in results:
        total += float(np.sum(r["out"].astype(np.float64)))
    return np.array(np.float32(total / (B * C)))


def kernel(x, labels, centers) -> np.ndarray:
    from concourse.bass_utils import run_bass_kernel_spmd

    nc = _get_nc()
    in_maps = make_in_maps(x, labels, centers)
    res = run_bass_kernel_spmd(nc, in_maps, list(range(NCORES)))
    return reduce_outputs(res.results)


# revision 12
# speedup vs baseline: 1.4074x; 1.4074x over previous
"""CenterLoss kernel for 8 Trainium2 NeuronCores.

loss = mean(distmat * onehot(labels)) over a (B, C) distmat where
distmat[i, j] = ||x_i - c_j||^2.  The mask selects exactly one element
per row, so  loss = (1/(B*C)) * sum_i ||x_i - c_{labels[i]}||^2.

Strategy: data-parallel over batch, 512 rows per core.  The gather of
512 center rows is Q7-emission-bound (~9ns/descriptor, ~4.6us serial
on the one GpSimd queue regardless of instruction count — dma_gather
measured identical per-descriptor cost plus an 8us library load, so
plain indirect DMAs win).  The pipeline is therefore shaped around the
serial emission:

  - x is loaded via SWDGE (gpsimd) first: it absorbs the one-time
    ~0.9us Q7 startup while the (HWDGE) index load is still in flight,
    so gather emission starts the moment the indices land.
  - gathers use DESCENDING chunk sizes 128,128,128,96,32: total
    emission time is fixed by descriptor count, but the last chunk's
    transfer + vector compute tail shrinks ~4x.
  - vector computes chunk n while chunk n+1 emits/transfers; per-chunk
    subtract + fused square-accumulate into a [128, 5] partial tile.
  - host sums the valid partials in float64 and divides by B*C.

Layout: chunk k / partition p holds batch row k*128 + p (host
pre-reorders x accordingly); chunk 3 uses partitions 0-95, chunk 4
partitions 0-31.
"""

import sys

if "/opt/trn_rl_repo" not in sys.path:
    sys.path.insert(0, "/opt/trn_rl_repo")

import numpy as np

import concourse.bass as bass
from concourse import mybir

NCORES = 8
B = 4096
D = 128
C = 20000
P = 128
BS = B // NCORES          # 512 rows per core
CHUNKS = [128, 128, 128, 96, 32]
NCH = len(CHUNKS)
assert sum(CHUNKS) == BS


def build_bass() -> bass.Bass:
    nc = bass.Bass(num_swdge_queues=2)
    x = nc.declare_dram_parameter("x", [P, NCH, D], mybir.dt.float32,
                                  isOutput=False)
    idx = nc.declare_dram_parameter("idx", [P, NCH], mybir.dt.int32,
                                    isOutput=False)
    centers = nc.declare_dram_parameter(
        "centers", [C, D], mybir.dt.float32, isOutput=False
    )
    out = nc.declare_dram_parameter("out", [P, NCH], mybir.dt.float32,
                                    isOutput=True)

    with (
        nc.sbuf_tensor([P, NCH], mybir.dt.int32) as idx_t,
        nc.sbuf_tensor([P, NCH, D], mybir.dt.float32) as x_t,
        nc.sbuf_tensor([P, NCH, D], mybir.dt.float32) as g_t,
        nc.sbuf_tensor([P, NCH, D], mybir.dt.float32) as d_t,
        nc.sbuf_tensor([P, NCH, D], mybir.dt.float32) as sq_t,
        nc.sbuf_tensor([P, NCH], mybir.dt.float32) as red_t,
        nc.semaphore("idx_sem") as idx_sem,
        nc.semaphore("x_sem") as x_sem,
        nc.semaphore("ga_sem") as ga_sem,
        nc.semaphore("gb_sem") as gb_sem,
        nc.semaphore("gc_sem") as gc_sem,
        nc.semaphore("gd_sem") as gd_sem,
        nc.semaphore("ge_sem") as ge_sem,
        nc.semaphore("m_sem") as m_sem,
        nc.semaphore("v_sem") as v_sem,
        nc.semaphore("done_sem") as done_sem,
    ):
        # Index load on HWDGE (Sync): issued first thing after the NEFF
        # preamble; its ~2us completion latency is hidden behind the
        # SWDGE startup + x-load emission below.
        idx_dma = nc.sync.dma_start(out=idx_t[:], in_=idx[:])
        idx_dma.ins.single_packet = True
        idx_dma.then_inc(idx_sem, 16)

        g_sems = [ga_sem, gb_sem, gc_sem, gd_sem, ge_sem]

        with nc.Block(no_gpsimd_drain=True) as block:

            @block.sync
            def _(sync):
                sync.wait_ge(v_sem, 2 * NCH)
                # Sync-queue drain at block end + NRT postamble quiesce
                # guarantee the store lands before kernel completion.
                out_dma = sync.dma_start(out=out[:], in_=red_t[:])
                out_dma.ins.single_packet = True
                out_dma.then_inc(done_sem, 16)

            @block.gpsimd
            def _(gpsimd):
                # Chunks 3/4 leave upper partitions of red_t untouched;
                # zero it so the full-tile out DMA reads defined data.
                # (Queue order: memset retires before the gathers issue,
                # and the vector STT follows the gather sems.)
                gpsimd.memset(red_t[:], 0.0).then_inc(m_sem, 1)
                # x load via SWDGE: absorbs the one-time Q7 startup
                # (~0.9us) and its emission finishes right as the idx
                # DMA completes, so the gathers below start immediately.
                gpsimd.dma_start(out=x_t[:], in_=x[:]).then_inc(x_sem, 16)
                gpsimd.wait_ge(idx_sem, 16)
                # One offset per partition per indirect DMA (HW limit),
                # so one gather per chunk; descending sizes shrink the
                # post-emission tail.  Alternate the two SWDGE queues so
                # transfers overlap (queue 0 first drains the x load).
                for k, rows in enumerate(CHUNKS):
                    gi = gpsimd.indirect_dma_start(
                        out=g_t[0:rows, k, :],
                        out_offset=None,
                        in_=centers[:],
                        in_offset=bass.IndirectOffsetOnAxis(
                            ap=idx_t[0:rows, k : k + 1], axis=0
                        ),
                    )
                    # alternate queues so transfers overlap (queue 0
                    # first drains the x load); one sem per DMA keeps
                    # the 16 per-engine increments unambiguous
                    if k % 2 == 0:
                        gi.ins.queue = "qPoolDynamic1"
                    gi.then_inc(g_sems[k], 16)

            @block.vector
            def _(vector):
                vector.wait_ge(m_sem, 1)
                vector.wait_ge(x_sem, 16)
                for k, rows in enumerate(CHUNKS):
                    vector.wait_ge(g_sems[k], 16)
                    vector.tensor_tensor(
                        out=d_t[0:rows, k, :],
                        in0=x_t[0:rows, k, :],
                        in1=g_t[0:rows, k, :],
                        op=mybir.AluOpType.subtract,
                    ).then_inc(v_sem, 1)
                    vector.wait_ge(v_sem, 2 * k + 1)
                    # sq = (d + 0) * d ; accum = sum(sq) — fused
                    # square+reduce
                    vector.scalar_tensor_tensor(
                        out=sq_t[0:rows, k, :],
                        in0=d_t[0:rows, k, :],
                        scalar=0.0,
                        in1=d_t[0:rows, k, :],
                        op0=mybir.AluOpType.add,
                        op1=mybir.AluOpType.mult,
                        accum_out=red_t[0:rows, k : k + 1],
                    ).then_inc(v_sem, 1)

    if not nc.is_finalized():
        nc.finalize()
    return nc


_NC = None


def _get_nc() -> bass.Bass:
    global _NC
    if _NC is None:
        _NC = build_bass()
    return _NC


# chunk/partition validity mask for the [P, NCH] partial tile
_MASK = np.zeros((P, NCH), dtype=bool)
for _k, _rows in enumerate(CHUNKS):
    _MASK[:_rows, _k] = True
_ROW_OF = np.full((P, NCH), -1, dtype=np.int64)
_off = 0
for _k, _rows in enumerate(CHUNKS):
    _ROW_OF[:_rows, _k] = _off + np.arange(_rows)
    _off += _rows


def make_in_maps(x, labels, centers):
    x = np.ascontiguousarray(np.asarray(x, dtype=np.float32))
    labels = np.asarray(labels).astype(np.int32)
    centers = np.ascontiguousarray(np.asarray(centers, dtype=np.float32))
    in_maps = []
    for c in range(NCORES):
        sl = slice(c * BS, (c + 1) * BS)
        xs = x[sl]
        lab = labels[sl]
        # chunk k / partition p <- shard row _ROW_OF[p, k]
        x_dev = np.zeros((P, NCH, D), dtype=np.float32)
        idx_dev = np.zeros((P, NCH), dtype=np.int32)
        x_dev[_MASK] = xs[_ROW_OF[_MASK]]
        idx_dev[_MASK] = lab[_ROW_OF[_MASK]]
        in_maps.append(
            {
                "x": np.ascontiguousarray(x_dev),
                "idx": np.ascontiguousarray(idx_dev),
                "centers": centers,
            }
        )
    return in_maps


def reduce_outputs(results) -> np.ndarray:
    total = 0.0
    for r in results:
        total += float(np.sum(r["out"][_MASK].astype(np.float64)))
    return np.array(np.float32(total / (B * C)))


def kernel(x, labels, centers) -> np.ndarray:
    from concourse.bass_utils import run_bass_kernel_spmd

    nc = _get_nc()
    in_maps = make_in_maps(x, labels, centers)
    res = run_bass_kernel_spmd(nc, in_maps, list(range(NCORES)))
    return reduce_outputs(res.results)


# revision 13
# speedup vs baseline: 1.4981x; 1.0645x over previous
"""CenterLoss kernel for 8 Trainium2 NeuronCores.

loss = mean(distmat * onehot(labels)) over a (B, C) distmat where
distmat[i, j] = ||x_i - c_j||^2.  The mask selects exactly one element
per row, so  loss = (1/(B*C)) * sum_i ||x_i - c_{labels[i]}||^2.

Strategy: data-parallel over batch.  Each of the 8 cores takes 512 rows
of x, gathers its 512 center rows from the (replicated) centers table
with 4 indirect DMAs (one per 128-row chunk, pipelined against the
vector engine), computes sum((x-g)^2) per chunk via subtract +
fused square-reduce (scalar_tensor_tensor accum), and writes a [128,4]
partial-sum tile.  The host sums the partials in float64 and divides
by B*C.

Raw Bass (no Tile): the toolchain allows at most one semaphore wait
per compute instruction, so cross-engine deps are taken with
standalone wait_ge instructions instead of instruction-attached waits.
"""

import sys

if "/opt/trn_rl_repo" not in sys.path:
    sys.path.insert(0, "/opt/trn_rl_repo")

import numpy as np

import concourse.bass as bass
from concourse import mybir

NCORES = 8
B = 4096
D = 128
C = 20000
P = 128
BS = B // NCORES          # 512 rows per core
N = BS // P               # 4 rows per partition


def build_bass() -> bass.Bass:
    nc = bass.Bass(num_swdge_queues=2)
    x = nc.declare_dram_parameter("x", [BS, D], mybir.dt.float32, isOutput=False)
    idx = nc.declare_dram_parameter("idx", [BS], mybir.dt.int32, isOutput=False)
    centers = nc.declare_dram_parameter(
        "centers", [C, D], mybir.dt.float32, isOutput=False
    )
    out = nc.declare_dram_parameter("out", [P, N], mybir.dt.float32, isOutput=True)

    with (
        nc.sbuf_tensor([P, N], mybir.dt.int32) as idx_t,
        nc.sbuf_tensor([P, N, D], mybir.dt.float32) as x_t,
        nc.sbuf_tensor([P, N, D], mybir.dt.float32) as g_t,
        nc.sbuf_tensor([P, N, D], mybir.dt.float32) as d_t,
        nc.sbuf_tensor([P, N, D], mybir.dt.float32) as sq_t,
        nc.sbuf_tensor([P, N], mybir.dt.float32) as red_t,
        nc.semaphore("idx_sem") as idx_sem,
        nc.semaphore("x_sem") as x_sem,
        nc.semaphore("ga_sem") as ga_sem,
        nc.semaphore("gb_sem") as gb_sem,
        nc.semaphore("gc_sem") as gc_sem,
        nc.semaphore("gd_sem") as gd_sem,
        nc.semaphore("v_sem") as v_sem,
        nc.semaphore("done_sem") as done_sem,
    ):
        g_sems = [ga_sem, gb_sem, gc_sem, gd_sem]

        # Issue the input loads in `main`, before the Block bodies: they
        # start earlier and their completion overlaps the block entry
        # overhead.  (Kept: the Block-end barrier is load-bearing — it
        # keeps the NRT per-engine postamble from contending with
        # in-flight gather completion semaphores.)
        idx_dma = nc.sync.dma_start(
            out=idx_t[:], in_=idx[:].rearrange("(p n) -> p n", p=P)
        )
        # single_packet measured inert for 128-partition transfers
        # (walrus falls back above the per-packet descriptor limit);
        # kept because it is harmless and correctness-verified.
        idx_dma.ins.single_packet = True
        idx_dma.then_inc(idx_sem, 16)
        nc.sync.dma_start(
            out=x_t[:], in_=x[:].rearrange("(p n) d -> p n d", p=P)
        ).then_inc(x_sem, 16)

        with nc.Block(no_gpsimd_drain=True) as block:

            @block.sync
            def _(sync):
                sync.wait_ge(v_sem, 2 * N)
                # No wait on done_sem: the Sync queue drain at block end
                # guarantees the store lands before kernel completion.
                out_dma = sync.dma_start(out=out[:], in_=red_t[:])
                out_dma.ins.single_packet = True
                out_dma.then_inc(done_sem, 16)

            @block.gpsimd
            def _(gpsimd):
                gpsimd.wait_ge(idx_sem, 16)
                # HW honors only one offset per partition per indirect
                # DMA, so issue N gathers with [P, 1] offset tiles.
                for n in range(N):
                    gi = gpsimd.indirect_dma_start(
                        out=g_t[:, n, :],
                        out_offset=None,
                        in_=centers[:],
                        in_offset=bass.IndirectOffsetOnAxis(
                            ap=idx_t[:, n : n + 1], axis=0
                        ),
                    )
                    # alternate the two SWDGE queues so transfers overlap
                    if n % 2 == 1:
                        gi.ins.queue = "qPoolDynamic1"
                    gi.then_inc(g_sems[n], 16)

            @block.vector
            def _(vector):
                vector.wait_ge(x_sem, 16)
                # Chunk n computes while chunk n+1's gather is in
                # flight.  The v_sem chain between dependent DVE ops is
                # cheap (it overlaps the per-op pipeline DRAIN) and
                # keeps the race detector happy.
                for n in range(N):
                    vector.wait_ge(g_sems[n], 16)
                    vector.tensor_tensor(
                        out=d_t[:, n, :],
                        in0=x_t[:, n, :],
                        in1=g_t[:, n, :],
                        op=mybir.AluOpType.subtract,
                    ).then_inc(v_sem, 1)
                    vector.wait_ge(v_sem, 2 * n + 1)
                    # sq = (d + 0) * d ; accum = sum(sq) — fused
                    # square+reduce
                    vector.scalar_tensor_tensor(
                        out=sq_t[:, n, :],
                        in0=d_t[:, n, :],
                        scalar=0.0,
                        in1=d_t[:, n, :],
                        op0=mybir.AluOpType.add,
                        op1=mybir.AluOpType.mult,
                        accum_out=red_t[:, n : n + 1],
                    ).then_inc(v_sem, 1)

    if not nc.is_finalized():
        nc.finalize()
    return nc


_NC = None


def _get_nc() -> bass.Bass:
    global _NC
    if _NC is None:
        _NC = build_bass()
    return _NC


def make_in_maps(x, labels, centers):
    x = np.ascontiguousarray(np.asarray(x, dtype=np.float32))
    labels = np.asarray(labels).astype(np.int32)
    centers = np.ascontiguousarray(np.asarray(centers, dtype=np.float32))
    in_maps = []
    for c in range(NCORES):
        sl = slice(c * BS, (c + 1) * BS)
        in_maps.append(
            {
                "x": np.ascontiguousarray(x[sl]),
                "idx": np.ascontiguousarray(labels[sl]),
                "centers": centers,
            }
        )
    return in_maps


def reduce_outputs(results) -> np.ndarray:
    total = 0.0
    for r in results:
        total += float(np.sum(r["out"].astype(np.float64)))
    return np.array(np.float32(total / (B * C)))


def kernel(x, labels, centers) -> np.ndarray:
    from concourse.bass_utils import run_bass_kernel_spmd

    nc = _get_nc()
    in_maps = make_in_maps(x, labels, centers)
    res = run_bass_kernel_spmd(nc, in_maps, list(range(NCORES)))
    return reduce_outputs(res.results)

